# revision 44
# baseline (speedup 1.0000x reference)
"""Multihead attention (B=2, S=2048, D=1024, 16 heads) on 8 trn2 NeuronCores.

Sharding: data-parallel over batch (2 groups of 4 cores), tensor-parallel over
heads within a group (4 heads/core, W_q/W_k/W_v column-sliced, W_o row-sliced).
Each core returns a partial [2048, 1024] output; the host sums the 4 partials
per batch and adds the constant row bv @ Wo + bo (the V-bias contribution is
constant because softmax rows sum to 1).

Key optimizations over the v1 kernel:
- Host-side KV packing: the key-padding mask drops ~half the keys, so the
  host gathers unmasked KV rows per batch and the kernel only computes
  scores/exp/PV over SKV_P ~= 1152 packed keys instead of 2048 (pad tiles
  carry zero V rows and a zero mask column, so they contribute nothing).
- bf16 operands for projections and attention matmuls (PSUM accumulation
  stays fp32); out-projection stays f32r.
- Paired-head score matmuls: the two heads of a pair live in PE row halves
  0-63 / 64-127 and their K=64 score matmuls execute concurrently via PE
  row tiling (tile_position auto-derived from base_partition).
- exp batches both heads of a pair in one ScalarE activation instruction
  ([128, 2, 512] across a 2-bank PSUM group, ping-ponged so ACT never
  stalls on the PE).
- PV / normalization / out-projection of the previous pair are interleaved
  between the score/exp groups of the current pair to keep PE dense while
  ACT (the bottleneck engine in the attention phase) streams exps.
"""

import math
import numpy as np

import concourse.bacc as bacc
import concourse.tile as tile
import concourse.mybir as mybir
from concourse.bass_utils import run_bass_kernel_spmd

F32 = mybir.dt.float32
F32R = mybir.dt.float32r
BF16 = mybir.dt.bfloat16
NP_BF16 = mybir.dt.np(BF16)
EXP = mybir.ActivationFunctionType.Exp
MULT = mybir.AluOpType.mult

B, SQ, SKV = 2, 2048, 2048
D, NH, HD = 1024, 16, 64
NCORES = 8
HPC = NH // (NCORES // B)     # 4 heads per core
CS = HPC * HD                 # 256 projection columns per core
QC = 512                      # q chunk
NQC = SQ // QC                # 4 q chunks
NDT = D // 128                # 8 contraction tiles
NST = SQ // 128               # 16 output row tiles

_SKV_P = None                 # packed kv length (multiple of 128), set on host


def _kchunks(skv_p):
    """Split skv_p into <=512 chunks for the K-projection PSUM tiles."""
    out = []
    off = 0
    while off < skv_p:
        c = min(512, skv_p - off)
        out.append((off, c))
        off += c
    return out


def _build(loop_n: int = 1, skv_p: int | None = None, variant: str = "full"):
    skv_p = skv_p if skv_p is not None else _SKV_P
    assert skv_p is not None and skv_p % 128 == 0
    nkt = skv_p // 128
    nc = bacc.Bacc(None, target_bir_lowering=False)
    xT = nc.dram_tensor("xT", [D, SQ], BF16, kind="ExternalInput")
    kvT = nc.dram_tensor("kvT", [D, skv_p], BF16, kind="ExternalInput")
    wq = nc.dram_tensor("wq", [128, NDT, CS], BF16, kind="ExternalInput")
    wk = nc.dram_tensor("wk", [128, NDT, CS], BF16, kind="ExternalInput")
    wv = nc.dram_tensor("wv", [128, NDT, CS], BF16, kind="ExternalInput")
    wo = nc.dram_tensor("wo", [128, 2, D], F32R, kind="ExternalInput")
    bqk = nc.dram_tensor("bqk", [128, 4], F32, kind="ExternalInput")
    mcol = nc.dram_tensor("mcol", [128, nkt], F32, kind="ExternalInput")
    out_p = nc.dram_tensor("out_p", [SQ, D], F32, kind="ExternalOutput")

    kch = _kchunks(skv_p)

    with tile.TileContext(nc) as tc:
        with tc.tile_pool(name="const", bufs=1) as const, \
             tc.tile_pool(name="big", bufs=1) as big:
            wq_sb = const.tile([128, NDT, CS], BF16)
            wk_sb = const.tile([128, NDT, CS], BF16)
            wv_sb = const.tile([128, NDT, CS], BF16)
            wo_sb = const.tile([128, 2, D], F32R)
            bqk_sb = const.tile([128, 4], F32)
            mcol_sb = const.tile([128, nkt], F32)
            ones4 = const.tile([128, HPC, 1], BF16)
            # wk gates the first matmul: put it at the head of the fast
            # HWDGE scalar ring, ahead of the kv tiles it will be used with
            nc.scalar.dma_start(out=wk_sb, in_=wk[:, :, :])
            nc.gpsimd.dma_start(out=wv_sb, in_=wv[:, :, :])
            nc.gpsimd.dma_start(out=wq_sb, in_=wq[:, :, :])
            nc.gpsimd.dma_start(out=bqk_sb, in_=bqk[:, :])
            nc.gpsimd.dma_start(out=mcol_sb, in_=mcol[:, :])
            nc.gpsimd.dma_start(out=wo_sb, in_=wo[:, :, :])
            nc.vector.memset(ones4, 1.0)

            QTs = {}
            for mh in range(2):
                for qc in range(NQC):
                    QTs[(mh, qc)] = big.tile([128, QC], BF16, tag=f"QT{mh}{qc}",
                                             name=f"QT{mh}{qc}")
            KT = big.tile([128, 2, skv_p], BF16)      # [hd(2x128), mh, kv]
            V = big.tile([128, nkt, HPC, HD + 1], BF16)  # V rows + mask col
            OT = big.tile([128, 2, SQ], F32R)         # [c(2x128), ct, q]

            if loop_n > 1:
                loop_cm = tc.For_i(0, loop_n, 1,
                                   hint_engines=(mybir.EngineType.PE,))
                loop_cm.__enter__()

            # ---- Phase 1: Q(qc0) + K + V projections; Q(qc1-3) runs as
            # attention fillers so ScalarE starts exping ~10us earlier ----
            xin_cm = tc.tile_pool(name="xin", bufs=1)
            xin = xin_cm.__enter__()
            kvin_cm = tc.tile_pool(name="kvin", bufs=1)
            kvin = kvin_cm.__enter__()
            if True:
                xts = []
                for dt in range(NDT):
                    xt_t = xin.tile([128, SQ], BF16, tag=f"xt{dt}",
                                    name=f"xt{dt}")
                    # qc0 slice first (gates Q0 proj), rest afterwards
                    nc.sync.dma_start(out=xt_t[:, 0:QC],
                                      in_=xT[dt * 128:(dt + 1) * 128, 0:QC])
                    xts.append(xt_t)
                kvts = []
                for dt in range(NDT):
                    kvt_t = kvin.tile([128, skv_p], BF16, tag=f"kv{dt}",
                                      name=f"kvt{dt}")
                    eng = nc.scalar if dt % 2 == 0 else nc.sync
                    eng.dma_start(out=kvt_t, in_=kvT[dt * 128:(dt + 1) * 128, :])
                    kvts.append(kvt_t)
                for dt in range(NDT):
                    nc.sync.dma_start(out=xts[dt][:, QC:SQ],
                                      in_=xT[dt * 128:(dt + 1) * 128, QC:SQ])

                with tc.tile_pool(name="pkv", bufs=1, space="PSUM") as pkv:
                    # Q(qc0): 2 banks, starts as soon as the qc0 xT slices land
                    psq0 = [pkv.tile([128, QC], F32, tag=f"q0{mh}",
                                     name=f"psq0{mh}") for mh in range(2)]
                    for dt in range(NDT):
                        for mh in range(2):
                            nc.tensor.matmul(psq0[mh],
                                             wq_sb[:, dt, mh * 128:(mh + 1) * 128],
                                             xts[dt][:, 0:QC],
                                             start=(dt == 0), stop=(dt == NDT - 1))
                    for mh in range(2):
                        nc.vector.tensor_scalar_add(
                            out=QTs[(mh, 0)],
                            in0=psq0[mh], scalar1=bqk_sb[:, mh:mh + 1])

                    # K^T: two dt-outer passes (one per mh) over 3 banks
                    psk = {ci: pkv.tile([128, cl], F32, tag=f"pk{ci}",
                                        name=f"psk{ci}")
                           for ci, (off, cl) in enumerate(kch)}
                    for mh in range(2):
                        for dt in range(NDT):
                            for ci, (off, cl) in enumerate(kch):
                                nc.tensor.matmul(psk[ci],
                                                 wk_sb[:, dt, mh * 128:(mh + 1) * 128],
                                                 kvts[dt][:, off:off + cl],
                                                 start=(dt == 0), stop=(dt == NDT - 1))
                        for ci, (off, cl) in enumerate(kch):
                            nc.vector.tensor_scalar_add(
                                out=KT[:, mh, off:off + cl],
                                in0=psk[ci],
                                scalar1=bqk_sb[:, 2 + mh:3 + mh])


            # ---- Phase 2: attention, software-pipelined over (qc, pair) ----
            from contextlib import ExitStack
            with ExitStack() as ph2:
                pp = ph2.enter_context(tc.tile_pool(name="pp", bufs=3))
                outp = ph2.enter_context(tc.tile_pool(name="outp", bufs=2))
                small = ph2.enter_context(tc.tile_pool(name="small", bufs=2))
                psc = ph2.enter_context(
                    tc.tile_pool(name="psc", bufs=2, space="PSUM"))
                pso = ph2.enter_context(
                    tc.tile_pool(name="pso", bufs=2, space="PSUM"))
                pout = ph2.enter_context(
                    tc.tile_pool(name="pout", bufs=1, space="PSUM"))
                pproj = ph2.enter_context(
                    tc.tile_pool(name="pproj", bufs=1, space="PSUM"))

                def pv_mms(P, pr, h, po_, t0, t1):
                    ph = 2 * pr + h
                    for t in range(t0, t1):
                        nc.tensor.matmul(po_, V[:, t, ph, :], P[:, h, t, :],
                                         start=(t == 0), stop=(t == nkt - 1))

                def norm(po_, h, pr, qsl):
                    po = h * 64
                    rec = small.tile([HD + 1, QC], F32, tag="rec", name="rec")
                    nc.vector.reciprocal(out=rec[HD:HD + 1, :],
                                         in_=po_[HD:HD + 1, :])
                    rec0 = small.tile([1, QC], F32, tag="rec0", name="rec0")
                    nc.scalar.dma_start(out=rec0[0:1, :], in_=rec[HD:HD + 1, :])
                    rb = small.tile([HD, QC], F32, tag="rb", name="rb")
                    nc.gpsimd.partition_broadcast(rb, rec0[0:1, :])
                    nc.vector.tensor_mul(out=OT[po:po + HD, pr, qsl],
                                         in0=po_[0:HD, :], in1=rb)

                def out_proj(st, alt=False):
                    ot_sb = outp.tile([128, D], F32, tag="osb", name="ot_sb")
                    for nk in range(2):
                        pol, tg = ((pproj, "pj") if (alt and nk == 1)
                                   else (pout, "po2"))
                        ps = pol.tile([128, QC], F32, tag=tg, name="ps_out")
                        for ct in range(2):
                            nc.tensor.matmul(ps,
                                             OT[:, ct, st * 128:(st + 1) * 128],
                                             wo_sb[:, ct, nk * QC:(nk + 1) * QC],
                                             start=(ct == 0), stop=(ct == 1))
                        nc.vector.tensor_copy(out=ot_sb[:, nk * QC:(nk + 1) * QC],
                                              in_=ps)
                        nc.sync.dma_start(
                            out=out_p[st * 128:(st + 1) * 128,
                                      nk * QC:(nk + 1) * QC],
                            in_=ot_sb[:, nk * QC:(nk + 1) * QC])

                # filler task queue: work from the previous (qc, pair)
                # iteration, emitted between this iteration's score groups so
                # the PE stays dense while ACT streams exps.
                filler = []

                def emit_filler(k):
                    for _ in range(k):
                        if filler:
                            filler.pop(0)()

                exp_q = QC if variant != "tiny_exp" else 64

                def qproj_task(mh, qc):
                    # two half-tasks (4 dt each) so filler granularity stays
                    # under ~1us; second half carries the bias drain
                    pb = pproj.tile([128, QC], F32, tag="pj", name="psqf")
                    def run_a(pb=pb, mh=mh, qc=qc):
                        for dt in range(NDT // 2):
                            nc.tensor.matmul(
                                pb, wq_sb[:, dt, mh * 128:(mh + 1) * 128],
                                xts[dt][:, qc * QC:(qc + 1) * QC],
                                start=(dt == 0), stop=False)
                    def run_b(pb=pb, mh=mh, qc=qc):
                        for dt in range(NDT // 2, NDT):
                            nc.tensor.matmul(
                                pb, wq_sb[:, dt, mh * 128:(mh + 1) * 128],
                                xts[dt][:, qc * QC:(qc + 1) * QC],
                                start=False, stop=(dt == NDT - 1))
                        nc.vector.tensor_scalar_add(
                            out=QTs[(mh, qc)], in0=pb,
                            scalar1=bqk_sb[:, mh:mh + 1])
                    return [run_a, run_b]

                def vproj_task(t, vp):
                    # full-width [128, 256] V tile per task (all 4 heads):
                    # half the PE instruction count of the per-pair variant
                    pool = pout if t % 2 == 0 else pproj
                    tg = "po2" if t % 2 == 0 else "pj"
                    pb = pool.tile([128, QC], F32, tag=tg, name=f"psv{t}{vp}")
                    def run(pb=pb, t=t):
                        ps = pb[:, 0:CS]
                        for dt in range(NDT):
                            nc.tensor.matmul(
                                ps, kvts[dt][:, t * 128:(t + 1) * 128],
                                wv_sb[:, dt, :],
                                start=(dt == 0), stop=(dt == NDT - 1))
                        nc.vector.tensor_scalar(
                            out=V[:, t, :, 0:HD],
                            in0=ps.rearrange("p (h d) -> p h d", h=HPC),
                            scalar1=mcol_sb[:, t:t + 1], scalar2=None, op0=MULT)
                        nc.vector.tensor_scalar(
                            out=V[:, t, :, HD:HD + 1], in0=ones4,
                            scalar1=mcol_sb[:, t:t + 1], scalar2=None, op0=MULT)
                    return run

                iters = [(qc, pr) for qc in range(NQC) for pr in range(2)]
                if variant == "phase1":
                    iters = []
                proj_sched = {}
                if iters:
                    proj_sched[0] = [vproj_task(t, 0) for t in range(nkt)]
                    for qcn in (1, 2, 3):
                        qts = []
                        for mh in range(2):
                            qts.extend(qproj_task(mh, qcn))
                        proj_sched[2 * qcn - 1] = (
                            proj_sched.get(2 * qcn - 1, []) + qts)
                for it_idx, (qc, pr) in enumerate(iters):
                    filler = proj_sched.get(it_idx, []) + filler
                    last = (qc, pr) == iters[-1]
                    qsl = slice(qc * QC, (qc + 1) * QC)
                    P = pp.tile([128, 2, nkt, QC], BF16, tag="P", name="P")
                    if last:
                        # chase tiles from pout + pproj (both idle during the
                        # last iteration) so they don't contend with the
                        # previous pair's PV slots in pso
                        po_pair = [pout.tile([HD + 1, QC], F32, tag="po2",
                                             name="po_"),
                                   pproj.tile([HD + 1, QC], F32, tag="pj",
                                              name="po_b")]
                    for t in range(nkt):
                        ss = psc.tile([128, 2, QC], F32, tag="ss", name="ss")
                        for h in range(2):
                            po = h * 64
                            nc.tensor.matmul(
                                ss[:, h, :],
                                KT[po:po + 64, pr, t * 128:(t + 1) * 128],
                                QTs[(pr, qc)][po:po + 64, :],
                                start=True, stop=True)
                        nc.scalar.activation(out=P[:, :, t, 0:exp_q],
                                             in_=ss[:, :, 0:exp_q],
                                             func=EXP, scale=0.125)
                        if last and t > 0:
                            # tail shrink: PV chases this pair's own exps
                            for h in range(2):
                                nc.tensor.matmul(po_pair[h], V[:, t - 1, 2 * pr + h, :],
                                                 P[:, h, t - 1, :],
                                                 start=(t == 1), stop=False)
                        emit_filler(2 if t > 0 else 0)

                    if last:
                        emit_filler(len(filler))
                        for h in range(2):
                            nc.tensor.matmul(po_pair[h], V[:, nkt - 1, 2 * pr + h, :],
                                             P[:, h, nkt - 1, :],
                                             start=(nkt == 1), stop=True)
                            norm(po_pair[h], h, pr, qsl)
                        for st in range(qc * NQC, (qc + 1) * NQC):
                            # tail out_proj draws accumulators from the
                            # drained psc slots (2 banks per st), leaving
                            # pout/pproj to the chase norms
                            ot_sb = outp.tile([128, D], F32, tag="osb",
                                              name="ot_sb")
                            pst = psc.tile([128, 2, QC], F32, tag="ss",
                                           name="ps_t")
                            for nk in range(2):
                                for ct in range(2):
                                    nc.tensor.matmul(
                                        pst[:, nk, :],
                                        OT[:, ct, st * 128:(st + 1) * 128],
                                        wo_sb[:, ct, nk * QC:(nk + 1) * QC],
                                        start=(ct == 0), stop=(ct == 1))
                                nc.vector.tensor_copy(
                                    out=ot_sb[:, nk * QC:(nk + 1) * QC],
                                    in_=pst[:, nk, :])
                                nc.sync.dma_start(
                                    out=out_p[st * 128:(st + 1) * 128,
                                              nk * QC:(nk + 1) * QC],
                                    in_=ot_sb[:, nk * QC:(nk + 1) * QC])
                        continue

                    # enqueue this pair's PV/norm (runs during next iteration)
                    def make_tasks(P=P, pr=pr, qsl=qsl, qc=qc):
                        ts = []
                        cl = max(2, (nkt + 3) // 4)
                        for h in range(2):
                            po_ = pso.tile([HD + 1, QC], F32, tag="po",
                                           name="po_")
                            for c0 in range(0, nkt, cl):
                                ts.append(lambda h=h, po_=po_, c0=c0:
                                          pv_mms(P, pr, h, po_, c0, min(c0 + cl, nkt)))
                            ts.append(lambda h=h, po_=po_: norm(po_, h, pr, qsl))
                        if pr == 1:
                            for st in range(qc * NQC, (qc + 1) * NQC):
                                ts.append(lambda st=st, alt=(qc >= 1):
                                          out_proj(st, alt))
                        return ts

                    filler.extend(make_tasks())
                # drain remaining work
                emit_filler(len(filler))

            kvin_cm.__exit__(None, None, None)
            xin_cm.__exit__(None, None, None)

            if loop_n > 1:
                loop_cm.__exit__(None, None, None)

    nc.compile()
    return nc


_NC = {}


def _get_nc(skv_p):
    if skv_p not in _NC:
        _NC[skv_p] = _build(skv_p=skv_p)
    return _NC[skv_p]


def _shard_inputs(query_input, key_value_input, key_padding_mask,
                  Wq, bq, Wk, bk, Wv, bv, Wo, bo):
    global _SKV_P
    keep = ~np.asarray(key_padding_mask)
    idxs = [np.nonzero(keep[b])[0] for b in range(B)]
    nmax = max(len(ix) for ix in idxs)
    skv_p = max(256, ((nmax + 127) // 128) * 128)
    _SKV_P = skv_p
    nkt = skv_p // 128

    in_maps = []
    for c in range(NCORES):
        b, hg = c // (NCORES // B), c % (NCORES // B)
        cs = slice(hg * CS, (hg + 1) * CS)
        ix = idxs[b]
        n = len(ix)
        kv_p = np.zeros((skv_p, D), np.float32)
        kv_p[:n] = key_value_input[b][ix]
        m01 = np.zeros((skv_p,), np.float32)
        m01[:n] = 1.0
        mcol = np.ascontiguousarray(m01.reshape(nkt, 128).T)  # [128, nkt]
        in_maps.append({
            "xT": np.ascontiguousarray(query_input[b].T).astype(NP_BF16),
            "kvT": np.ascontiguousarray(kv_p.T).astype(NP_BF16),
            "wq": np.ascontiguousarray(
                Wq[:, cs].reshape(NDT, 128, CS).transpose(1, 0, 2)).astype(NP_BF16),
            "wk": np.ascontiguousarray(
                Wk[:, cs].reshape(NDT, 128, CS).transpose(1, 0, 2)).astype(NP_BF16),
            "wv": np.ascontiguousarray(
                Wv[:, cs].reshape(NDT, 128, CS).transpose(1, 0, 2)).astype(NP_BF16),
            "wo": np.ascontiguousarray(
                Wo[cs, :].reshape(2, 128, D).transpose(1, 0, 2)),
            "bqk": np.ascontiguousarray(
                np.stack([bq[cs][:128], bq[cs][128:],
                          bk[cs][:128], bk[cs][128:]], axis=1)),
            "mcol": mcol,
        })
    return in_maps


def kernel(query_input, key_value_input, key_padding_mask,
           Wq, bq, Wk, bk, Wv, bv, Wo, bo):
    query_input = np.asarray(query_input, np.float32)
    key_value_input = np.asarray(key_value_input, np.float32)
    key_padding_mask = np.asarray(key_padding_mask)
    Wq = np.asarray(Wq, np.float32); bq = np.asarray(bq, np.float32)
    Wk = np.asarray(Wk, np.float32); bk = np.asarray(bk, np.float32)
    Wv = np.asarray(Wv, np.float32); bv = np.asarray(bv, np.float32)
    Wo = np.asarray(Wo, np.float32); bo = np.asarray(bo, np.float32)

    in_maps = _shard_inputs(query_input, key_value_input, key_padding_mask,
                            Wq, bq, Wk, bk, Wv, bv, Wo, bo)
    nc = _get_nc(_SKV_P)
    res = run_bass_kernel_spmd(nc, in_maps, core_ids=list(range(NCORES)))

    # unshard: sum the 4 row-parallel partials per batch; V-bias contributes a
    # constant row (softmax rows sum to 1) folded in with bo here.
    const_row = (bv.astype(np.float64) @ Wo.astype(np.float64)) + bo.astype(np.float64)
    gpc = NCORES // B
    out = np.empty((B, SQ, D), np.float32)
    for b in range(B):
        acc = np.zeros((SQ, D), np.float64)
        for hg in range(gpc):
            acc += res.results[b * gpc + hg]["out_p"].astype(np.float64)
        out[b] = (acc + const_row[None, :]).astype(np.float32)
    return out
